# revision 1
# baseline (speedup 1.0000x reference)
"""Trainium2 Bass kernel for varlen (ragged) BERT self-attention.

Strategy: tensor-parallel over heads. 16 heads across 8 NeuronCores ->
2 heads per core. Every core runs an IDENTICAL program (SPMD) on:
  - xt:   full hidden_states, pre-transposed+cast to bf16 on host, (1024, nnz)
  - wt:   this core's slice of Wqkv (q/k/v rows of its 2 heads), as
          matmul-lhsT blocks (8, 128, 384) bf16
  - bias: this core's bias slice (3, 128) f32
Output per core: (nnz, 128) f32 = the 2 owned heads' output columns.
Host concatenates core outputs along axis 1.

On-chip per core:
  1. QKV projection: Y^T[384, nnz] = Wc @ X^T, K=1024 in 8 chunks,
     bias added during PSUM->SBUF eviction (DVE tensor_scalar add),
     cast bf16. Gives qT/kT/vT resident in SBUF as [128(=2hx64), nnz].
  2. Attention per "unit" (a sequence, or a pack of small consecutive
     sequences), per head: scoresT[k,q] = kT.T @ qT (K=64); for packs a
     second rank-4 matmul accumulates -10000 into cross-sequence score
     entries (mask rows mk/mq) so exp underflows to zero. exp on ACT
     (1/sqrt(64) folded into the activation scale), then out^T[65, q]
     accumulated as (v|ones).T @ expT -- the ones column yields the
     softmax denominator for free. PE-transpose + per-partition
     reciprocal*mul normalizes and lands (tok, 64) f32 blocks.

Emission order interleaves per-unit attention into the QKV chunk
stream (chunks processed back-to-front, units become ready
largest-first) so the PE instruction stream stays dense end-to-end --
otherwise the HAM clock gate re-throttles the tensor engine to half
clock during attention-heavy stretches.

No padding: every sequence is processed at its true length.
"""

import functools
import sys

import numpy as np

for _p in ("/opt/trn_rl_repo",):
    if _p not in sys.path:
        sys.path.append(_p)

import ml_dtypes  # noqa: E402

N_HEADS = 16
HEAD_DIM = 64
DIM = 1024
N_CORES = 8
HEADS_PER_CORE = N_HEADS // N_CORES  # 2

PACK_MAX_LEN = 768  # pack adjacent seqs <=512 up to this many tokens
PACK_MAX_SEQS = 4  # rank of the additive mask term


def _make_units(lengths):
    """Group sequences into attention units: [(offset, L, [seq len list])]."""
    units = []
    off = 0
    cur = None  # (start, [lens])
    for L in lengths:
        if L == 0:
            continue
        if L <= 512:
            if (
                cur is not None
                and sum(cur[1]) + L <= PACK_MAX_LEN
                and len(cur[1]) < PACK_MAX_SEQS
            ):
                cur[1].append(L)
            else:
                if cur is not None:
                    units.append((cur[0], sum(cur[1]), list(cur[1])))
                cur = (off, [L])
        else:
            if cur is not None:
                units.append((cur[0], sum(cur[1]), list(cur[1])))
                cur = None
            units.append((off, L, [L]))
        off += L
    if cur is not None:
        units.append((cur[0], sum(cur[1]), list(cur[1])))
    return units


@functools.lru_cache(maxsize=4)
def _build(nnz, lengths):
    """Build + compile the SPMD Bass program for the given ragged lengths."""
    import concourse.mybir as mybir
    import concourse.tile as tile
    from concourse import bacc
    from concourse.masks import make_identity

    f32 = mybir.dt.float32
    bf16 = mybir.dt.bfloat16
    Exp = mybir.ActivationFunctionType.Exp

    KC = DIM // 128  # 8 contraction chunks
    M3 = 3 * HEADS_PER_CORE * HEAD_DIM  # 384 output dims per core
    D = HEAD_DIM
    HP = HEADS_PER_CORE

    nc = bacc.Bacc("TRN2", target_bir_lowering=False, debug=False)
    xt = nc.declare_dram_parameter("xt", [DIM, nnz], bf16, isOutput=False)
    wt = nc.declare_dram_parameter("wt", [KC, 128, M3], bf16, isOutput=False)
    bias = nc.declare_dram_parameter("bias", [3, 128], f32, isOutput=False)
    out = nc.declare_dram_parameter("out", [nnz, 128], f32, isOutput=True)

    units = _make_units(lengths)
    n_tok_chunks = (nnz + 511) // 512

    with tile.TileContext(nc) as tc:
        with (
            tc.tile_pool(name="res", bufs=1) as res,
            tc.tile_pool(name="xp", bufs=4) as xp,
            tc.tile_pool(name="esp", bufs=6) as esp,
            tc.tile_pool(name="vgp", bufs=20) as vgp,
            tc.tile_pool(name="osp", bufs=3) as osp,
            tc.tile_pool(name="rsp", bufs=3) as rsp,
            tc.tile_pool(name="obp", bufs=4) as obp,
            tc.tile_pool(name="ps", bufs=1, space="PSUM") as ps,
        ):
            # --- constants / resident tensors ---
            wt_sb = res.tile([128, KC, M3], bf16)
            nc.sync.dma_start(wt_sb[:], wt[:, :, :].rearrange("a p m -> p a m"))
            bias_sb = res.tile([128, 3], f32)
            nc.sync.dma_start(bias_sb[:], bias[:, :].rearrange("a p -> p a"))
            ident_bf = res.tile([128, 128], bf16)
            make_identity(nc, ident_bf[:])
            ident_f32 = res.tile([128, 128], f32)
            make_identity(nc, ident_f32[:])

            qT = res.tile([128, nnz], bf16)
            kT = res.tile([128, nnz], bf16)
            vT = res.tile([128, nnz], bf16)
            qkvT = (qT, kT, vT)

            # persistent v_aug slots: [ktok(128), v(64)+ones(1)]; the ones
            # column is written once, the v part is refreshed per unit
            max_nk = max((u[1] + 127) // 128 for u in units)
            va_slots = {}
            for h in range(HP):
                for jc in range(max_nk):
                    va = res.tile([128, D + 1], bf16, name=f"va{h}_{jc}")
                    nc.gpsimd.memset(va[:, D : D + 1], 1.0)
                    va_slots[(h, jc)] = va

            # --- pack mask rows: score += sum_r mk[r,i] * mq[r,j] ---
            # mk[r,i] = 100 on pack-local seq r's keys, else 0
            # mq[r,j] = 0 on pack-local seq r's queries, else -100
            # => cross-sequence entries within a pack get -10000.
            has_packs = any(len(u[2]) > 1 for u in units)
            if has_packs:
                # 32 partitions for gpsimd alignment; matmuls read rows 0:4
                mk = res.tile([32, nnz], bf16)
                mq = res.tile([32, nnz], bf16)
                nc.gpsimd.memset(mk[:, :], 0.0)
                nc.gpsimd.memset(mq[:, :], 0.0)
                for O, Lp, ls in units:
                    if len(ls) < 2:
                        continue
                    nc.gpsimd.memset(mq[:, O : O + Lp], -100.0)
                    so = O
                    for r, L in enumerate(ls):
                        # row r gets 100 (mk) / 0 (mq) on seq r's columns:
                        # predicate (partition - r) != 0 keeps old value
                        nc.gpsimd.affine_select(
                            out=mk[:, so : so + L],
                            in_=mk[:, so : so + L],
                            compare_op=mybir.AluOpType.not_equal,
                            fill=100.0,
                            base=-r,
                            pattern=[[0, L]],
                            channel_multiplier=1,
                        )
                        nc.gpsimd.affine_select(
                            out=mq[:, so : so + L],
                            in_=mq[:, so : so + L],
                            compare_op=mybir.AluOpType.not_equal,
                            fill=0.0,
                            base=-r,
                            pattern=[[0, L]],
                            channel_multiplier=1,
                        )
                        so += L

            xt_view = xt[:, :].rearrange("(a p) n -> p a n", p=128)

            # --- QKV feeder: yields one (ti, mc) matmul group at a time so
            # attention emission can interleave dense PE work (keeps the HAM
            # clock gate released during ACT-bound attention stretches) ---
            state = {"ti_next": n_tok_chunks}  # smallest fully-emitted chunk

            def _qkv_groups():
                for ti in range(n_tok_chunks - 1, -1, -1):
                    t0 = ti * 512
                    nt = min(512, nnz - t0)
                    xt_tile = xp.tile([128, KC, 512], bf16, tag="xt", name="xt_t")
                    nc.sync.dma_start(
                        xt_tile[:, :, :nt], xt_view[:, :, t0 : t0 + nt]
                    )
                    for mc in range(3):
                        mm = ps.tile([128, 512], f32, tag="mm", bufs=1, name="mm")
                        for kc in range(KC):
                            nc.tensor.matmul(
                                mm[:, :nt],
                                wt_sb[:, kc, mc * 128 : (mc + 1) * 128],
                                xt_tile[:, kc, :nt],
                                start=(kc == 0),
                                stop=(kc == KC - 1),
                            )
                        # evict + bias + cast on DVE
                        nc.vector.tensor_scalar_add(
                            qkvT[mc][:, t0 : t0 + nt],
                            mm[:, :nt],
                            bias_sb[:, mc : mc + 1],
                        )
                        if mc == 2:
                            state["ti_next"] = ti
                        yield

            feeder = _qkv_groups()

            # pacing: spread remaining feeder groups over remaining
            # attention jc-iterations (recomputed each step)
            n_groups = 3 * n_tok_chunks
            n_iters = sum(
                ((u[1] + 127) // 128) * ((u[1] + 511) // 512) for u in units
            )
            pace = {"acc": 0.0, "groups": n_groups, "iters": n_iters}

            def feed(n):
                for _ in range(n):
                    if next(feeder, "done") == "done":
                        break
                    pace["groups"] -= 1

            def feed_cb():
                if pace["iters"] > 0:
                    pace["acc"] += pace["groups"] / pace["iters"]
                pace["iters"] -= 1
                k = min(int(pace["acc"]), pace["groups"])
                if k > 0:
                    pace["acc"] -= k
                    feed(k)
                elif pace["groups"] == 0:
                    # feeder dry: emit PE keepalive matmuls so the HAM clock
                    # gate stays released through the ACT-bound tail
                    for _ in range(2):
                        dm = ps.tile([128, 512], f32, tag="mm", bufs=1, name="dm")
                        nc.tensor.matmul(
                            dm[:, :],
                            wt_sb[:, 0, 0:128],
                            qT[:, 0:512],
                            start=True,
                            stop=True,
                        )

            def emit_attention(O, L, ls):
                masked = len(ls) > 1
                nk = (L + 127) // 128
                nq5 = (L + 511) // 512
                # pack-local seq boundaries for block-sparse skipping
                bounds = []
                so = 0
                for sl in ls:
                    bounds.append((so, so + sl))
                    so += sl

                def seqs_in(a, b):
                    return {
                        i
                        for i, (s0, s1) in enumerate(bounds)
                        if a < s1 and b > s0
                    }
                # refresh v_aug slots (v natural), both heads
                for h in range(HP):
                    p0 = D * h
                    for jc in range(nk):
                        nj = min(128, L - jc * 128)
                        vps = ps.tile([128, D], bf16, tag="tp", bufs=1, name="vps")
                        nc.tensor.transpose(
                            vps[:nj, :D],
                            vT[p0 : p0 + D, O + jc * 128 : O + jc * 128 + nj],
                            ident_bf[p0 : p0 + D, p0 : p0 + D],
                        )
                        nc.vector.tensor_copy(
                            va_slots[(h, jc)][:nj, 0:D], vps[:nj, :D]
                        )
                for qc in range(nq5):
                    q0 = qc * 512
                    nq = min(512, L - q0)
                    ovs = [
                        ps.tile([D + 1, 512], f32, tag="ov", bufs=2, name=f"ov{h}")
                        for h in range(HP)
                    ]
                    qseqs = seqs_in(q0, q0 + nq)
                    active = [
                        jc
                        for jc in range(nk)
                        if seqs_in(jc * 128, min(jc * 128 + 128, L)) & qseqs
                    ]
                    pairs = [active[i : i + 2] for i in range(0, len(active), 2)]
                    for pair in pairs:
                        feed_cb()
                        for h in range(HP):
                            p0 = D * h
                            sps = ps.tile(
                                [128, 2, 512], f32, tag="sc", bufs=2, name="sps"
                            )
                            es = esp.tile(
                                [128, 2, 512], bf16, tag="es", name="es"
                            )
                            njs = []
                            for sl, jc in enumerate(pair):
                                nj = min(128, L - jc * 128)
                                njs.append(nj)
                                kseqs = seqs_in(jc * 128, jc * 128 + nj)
                                need_mask = masked and not (
                                    len(kseqs) == 1 and kseqs == qseqs
                                )
                                nc.tensor.matmul(
                                    sps[:nj, sl, :nq],
                                    kT[
                                        p0 : p0 + D,
                                        O + jc * 128 : O + jc * 128 + nj,
                                    ],
                                    qT[p0 : p0 + D, O + q0 : O + q0 + nq],
                                    start=True,
                                    stop=not need_mask,
                                )
                                if need_mask:
                                    nc.tensor.matmul(
                                        sps[:nj, sl, :nq],
                                        mk[:, O + jc * 128 : O + jc * 128 + nj],
                                        mq[:, O + q0 : O + q0 + nq],
                                        start=False,
                                        stop=True,
                                    )
                            nja = max(njs)
                            if len(pair) == 2:
                                nc.scalar.activation(
                                    es[:nja, :, :nq],
                                    sps[:nja, :, :nq],
                                    Exp,
                                    scale=0.125,
                                )
                            else:
                                nc.scalar.activation(
                                    es[:nja, 0, :nq],
                                    sps[:nja, 0, :nq],
                                    Exp,
                                    scale=0.125,
                                )
                            for sl, jc in enumerate(pair):
                                nj = njs[sl]
                                nc.tensor.matmul(
                                    ovs[h][:, :nq],
                                    va_slots[(h, jc)][:nj, :],
                                    es[:nj, sl, :nq],
                                    start=(jc == active[0]),
                                    stop=(jc == active[-1]),
                                )
                    for h in range(HP):
                        p0 = D * h
                        nfull = nq // 128
                        nrem = nq - nfull * 128
                        osb = osp.tile([D + 1, 512], f32, tag="os", name="osb")
                        nc.vector.tensor_copy(osb[:, :nq], ovs[h][:, :nq])
                        ob = obp.tile([128, 4, D], f32, tag="ob", name="ob")
                        for q1 in range((nq + 127) // 128):
                            nqq = min(128, nq - q1 * 128)
                            tps = ps.tile(
                                [128, D + 1], f32, tag="tp", bufs=1, name="tps"
                            )
                            nc.tensor.transpose(
                                tps[:nqq, :],
                                osb[:, q1 * 128 : q1 * 128 + nqq],
                                ident_f32[0 : D + 1, 0 : D + 1],
                            )
                            rs = rsp.tile([128, 1], f32, tag="rs", name="rs")
                            nc.vector.reciprocal(rs[:nqq, :], tps[:nqq, D : D + 1])
                            nc.vector.tensor_scalar_mul(
                                ob[:nqq, q1, :], tps[:nqq, 0:D], rs[:nqq, :]
                            )
                        if nfull:
                            nc.sync.dma_start(
                                out[O + q0 : O + q0 + nfull * 128, p0 : p0 + D]
                                .rearrange("(a p) c -> p a c", p=128),
                                ob[:, :nfull, :],
                            )
                        if nrem:
                            nc.sync.dma_start(
                                out[
                                    O + q0 + nfull * 128 : O + q0 + nq,
                                    p0 : p0 + D,
                                ],
                                ob[:nrem, nfull, :],
                            )

            # --- interleaved emission ---
            # chunks back-to-front via the feeder; a unit is ready once all
            # chunks covering [O, O+L) are emitted, i.e. O >= 512*ti_next.
            # Attention units then pull more feeder groups as they emit.
            pending = sorted(units, key=lambda u: u[0], reverse=True)
            pack_idx = [i for i, u in enumerate(pending) if len(u[2]) > 1]
            if pack_idx and pack_idx[0] > 0:
                # move the unit just before the first pack to the very end:
                # its chunks are long emitted, so it gives the tail (which
                # has no feeder filler left) independent PE work
                tail_u = pending.pop(pack_idx[0] - 1)
                pending.append(tail_u)
            for u in pending:
                while state["ti_next"] * 512 > u[0]:
                    feed(1)
                emit_attention(*u)
            feed(n_groups)  # drain any leftovers

    nc.compile()
    return nc


def _prepare(hidden_states, Wqkv_weight, Wqkv_bias, cu_seqlens):
    """Host-side sharding prep. Returns (nc, in_maps)."""
    hs = np.asarray(hidden_states, dtype=np.float32)
    W = np.asarray(Wqkv_weight, dtype=np.float32)
    b = np.asarray(Wqkv_bias, dtype=np.float32).reshape(-1)
    cs = np.asarray(cu_seqlens).astype(np.int64).reshape(-1)
    nnz, dim = hs.shape
    assert dim == DIM and W.shape == (3 * DIM, DIM)
    lengths = tuple(int(cs[i + 1] - cs[i]) for i in range(len(cs) - 1))
    assert sum(lengths) == nnz, (lengths, nnz)

    nc = _build(nnz, lengths)

    xt_np = np.ascontiguousarray(hs.T).astype(ml_dtypes.bfloat16)
    in_maps = []
    for c in range(N_CORES):
        r0 = c * HEADS_PER_CORE * HEAD_DIM  # 128c
        rows = []
        biases = []
        for part in range(3):  # q, k, v
            rows.append(W[part * DIM + r0 : part * DIM + r0 + 128, :])
            biases.append(b[part * DIM + r0 : part * DIM + r0 + 128])
        Wc = np.concatenate(rows, axis=0)  # (384, 1024)
        wt_np = np.ascontiguousarray(Wc.T.reshape(DIM // 128, 128, 384)).astype(
            ml_dtypes.bfloat16
        )
        bias_np = np.ascontiguousarray(np.stack(biases, axis=0))  # (3, 128)
        in_maps.append({"xt": xt_np, "wt": wt_np, "bias": bias_np})
    return nc, in_maps


def kernel(hidden_states, Wqkv_weight, Wqkv_bias, cu_seqlens, max_seqlen=None):
    from concourse.bass_utils import run_bass_kernel_spmd

    nc, in_maps = _prepare(hidden_states, Wqkv_weight, Wqkv_bias, cu_seqlens)
    res = run_bass_kernel_spmd(nc, in_maps, list(range(N_CORES)))
    out = np.concatenate(
        [res.results[c]["out"] for c in range(N_CORES)], axis=1
    )
    return np.ascontiguousarray(out, dtype=np.float32)



# revision 2
# speedup vs baseline: 1.3677x; 1.3677x over previous
"""Trainium2 Bass kernel for varlen (ragged) BERT self-attention.

Strategy: tensor-parallel over heads. 16 heads across 8 NeuronCores ->
2 heads per core. Every core runs an IDENTICAL program (SPMD) on:
  - xt:   full hidden_states, host-padded so each sequence starts at a
          128-aligned token offset, transposed + bf16, laid out
          [128, n_ti, 8, 512] so each 512-token chunk is one contiguous
          8KB-per-partition DMA read.
  - wqk:  this core's q/k weight slices as matmul-lhsT blocks
          [8, 128, 256] bf16 (cols 0:128 = q, 128:256 = k).
  - wv:   this core's v weight slice as matmul-rhs blocks [8, 128, 128].
  - bias: q/k bias [128, 2] f32 (per-partition adds on eviction).
Output per core: raw [130, nnz_pad] f32: rows 0:64   = sum_k p*v (head0)
                                         row  64    = sum_k p   (head0)
                                         rows 65:129/129 same for head1.
Host divides by the denominator row, adds the v bias (algebraically
exact: softmax(s) @ (v+bv) = softmax(s)@v + bv), and gathers the
padded layout back to the ragged one.

On-chip per core:
  1. QKV projection, K=1024 in 8 chunks of 128:
     - q,k computed TRANSPOSED (stationary = W slices): qT/kT
       [128(=2h x 64d), nnz_pad] bf16, bias added during PSUM->SBUF
       eviction (DVE tensor_scalar add, per-partition).
     - v computed NATURAL (stationary = xt chunks, moving = Wv):
       v_nat [128(tok), chunk, 2, 65] bf16 with a preset ones column;
       no transposes needed anywhere.
  2. Attention per sequence (no packing; every sequence 128-aligned):
     scores[k,q] = kT.T @ qT per head with K=64 -> the two heads'
     matmuls auto-place on PE row tiles (0,0)/(64,0) and run
     CONCURRENTLY. exp on ACT over both heads in one instruction
     (scale=1/8 folded in). outT[65, q] accumulated as
     (v|ones).T @ exp with K<=128; the ones column gives the softmax
     denominator for free. PSUM -> SBUF copy (DVE) -> DMA out raw.

Emission interleaves per-jc attention into the QKV chunk stream
(chunks processed back-to-front, sequences emitted largest-first) so
the PE instruction stream stays dense end-to-end (HAM clock gate).
"""

import functools
import sys

import numpy as np

for _p in ("/opt/trn_rl_repo",):
    if _p not in sys.path:
        sys.path.append(_p)

import ml_dtypes  # noqa: E402

N_HEADS = 16
HEAD_DIM = 64
DIM = 1024
N_CORES = 8
HEADS_PER_CORE = N_HEADS // N_CORES  # 2


def _padded_units(lengths):
    """One unit per sequence at a 128-aligned padded offset."""
    units = []
    off = 0
    for L in lengths:
        if L == 0:
            continue
        units.append((off, L))
        off += ((L + 127) // 128) * 128
    nnz_pad = ((off + 511) // 512) * 512
    return tuple(units), nnz_pad


@functools.lru_cache(maxsize=4)
def _build(nnz_pad, units):
    """Build + compile the SPMD Bass program for the given ragged lengths."""
    import concourse.mybir as mybir
    import concourse.tile as tile
    from concourse import bacc

    f32 = mybir.dt.float32
    bf16 = mybir.dt.bfloat16
    Exp = mybir.ActivationFunctionType.Exp

    KC = DIM // 128  # 8 contraction chunks
    D = HEAD_DIM
    n_ti = nnz_pad // 512
    n_ch = nnz_pad // 128

    nc = bacc.Bacc("TRN2", target_bir_lowering=False, debug=False)
    xt = nc.declare_dram_parameter("xt", [128, n_ti, KC, 512], bf16, isOutput=False)
    wqk = nc.declare_dram_parameter("wqk", [KC, 128, 256], bf16, isOutput=False)
    wv = nc.declare_dram_parameter("wv", [KC, 128, 128], bf16, isOutput=False)
    bias = nc.declare_dram_parameter("bias", [128, 2], f32, isOutput=False)
    out = nc.declare_dram_parameter("out", [130, nnz_pad], f32, isOutput=True)

    with tile.TileContext(nc) as tc:
        with (
            tc.tile_pool(name="res", bufs=1) as res,
            tc.tile_pool(name="xp", bufs=4) as xp,
            tc.tile_pool(name="esp", bufs=3) as esp,
            tc.tile_pool(name="otp", bufs=4) as otp,
            tc.tile_pool(name="ps", bufs=1, space="PSUM") as ps,
        ):
            # --- resident tensors ---
            wqk_sb = res.tile([128, KC, 256], bf16)
            nc.sync.dma_start(wqk_sb[:], wqk[:, :, :].rearrange("a p m -> p a m"))
            wv_sb = res.tile([128, KC, 128], bf16)
            nc.sync.dma_start(wv_sb[:], wv[:, :, :].rearrange("a p m -> p a m"))
            bias_sb = res.tile([128, 2], f32)
            nc.sync.dma_start(bias_sb[:], bias[:, :])

            qT = res.tile([128, nnz_pad], bf16)
            kT = res.tile([128, nnz_pad], bf16)
            # v in natural layout: [tok(128), chunk, head, 64 v + 1 ones]
            v_nat = res.tile([128, n_ch, 2, D + 1], bf16)
            nc.gpsimd.memset(v_nat[:, :, :, D : D + 1], 1.0)

            # --- QKV feeder: yields 3 groups per token chunk (q, k+v0, v1)
            # so attention emission can interleave dense PE work ---
            state = {"ti_next": n_ti}

            def _qkv_groups():
                for ti in range(n_ti - 1, -1, -1):
                    t0 = ti * 512
                    xt_tile = xp.tile([128, KC, 512], bf16, tag="xt", name="xt_t")
                    nc.sync.dma_start(xt_tile[:], xt[:, ti, :, :])
                    # q group: stationary W, moving xt
                    mmq = ps.tile([128, 512], f32, tag="mm", bufs=2, name="mmq")
                    for kc in range(KC):
                        nc.tensor.matmul(
                            mmq[:, :],
                            wqk_sb[:, kc, 0:128],
                            xt_tile[:, kc, :],
                            start=(kc == 0),
                            stop=(kc == KC - 1),
                        )
                    nc.vector.tensor_scalar_add(
                        qT[:, t0 : t0 + 512], mmq[:, :], bias_sb[:, 0:1]
                    )
                    yield
                    # k group interleaved with first half of v (v: stationary
                    # xt chunk, moving Wv -> LDW-heavy; hide under k streams)
                    mmk = ps.tile([128, 512], f32, tag="mm", bufs=2, name="mmk")
                    mmv = ps.tile([128, 4, 2, D], f32, tag="mm", bufs=2, name="mmv")
                    vs = [(c, kc) for c in range(4) for kc in range(KC)]
                    vi = 0

                    def _vmm(c, kc):
                        nc.tensor.matmul(
                            mmv[:, c, :, :],
                            xt_tile[:, kc, c * 128 : (c + 1) * 128],
                            wv_sb[:, kc, :],
                            start=(kc == 0),
                            stop=(kc == KC - 1),
                        )

                    for kc in range(KC):
                        nc.tensor.matmul(
                            mmk[:, :],
                            wqk_sb[:, kc, 128:256],
                            xt_tile[:, kc, :],
                            start=(kc == 0),
                            stop=(kc == KC - 1),
                        )
                        for _ in range(2):
                            _vmm(*vs[vi])
                            vi += 1
                    nc.vector.tensor_scalar_add(
                        kT[:, t0 : t0 + 512], mmk[:, :], bias_sb[:, 1:2]
                    )
                    yield
                    # rest of v
                    while vi < len(vs):
                        _vmm(*vs[vi])
                        vi += 1
                    nc.vector.tensor_copy(
                        v_nat[:, ti * 4 : ti * 4 + 4, :, 0:D], mmv[:, :, :, :]
                    )
                    state["ti_next"] = ti
                    yield

            feeder = _qkv_groups()

            # pacing: spread remaining feeder groups over remaining
            # attention jc-iterations (recomputed each step)
            n_groups = 3 * n_ti
            n_iters = sum(
                ((L + 127) // 128) * ((L + 511) // 512) for _, L in units
            )
            pace = {"acc": 0.0, "groups": n_groups, "iters": n_iters}

            def feed(n):
                for _ in range(n):
                    if next(feeder, "done") == "done":
                        break
                    pace["groups"] -= 1

            def feed_cb():
                if pace["iters"] > 0:
                    pace["acc"] += pace["groups"] / pace["iters"]
                pace["iters"] -= 1
                k = min(int(pace["acc"]), pace["groups"])
                if k > 0:
                    pace["acc"] -= k
                    feed(k)
                elif pace["groups"] == 0:
                    # feeder dry: PE keepalive so the HAM clock gate stays
                    # released through ACT-bound stretches
                    for _ in range(2):
                        dm = ps.tile([128, 512], f32, tag="mm", bufs=2, name="dm")
                        nc.tensor.matmul(
                            dm[:, :],
                            wqk_sb[:, 0, 0:128],
                            qT[:, 0:512],
                            start=True,
                            stop=True,
                        )

            def emit_attention(O, L):
                gb = O // 128
                nk = (L + 127) // 128
                for qc in range((L + 511) // 512):
                    q0 = O + qc * 512
                    nq = min(512, L - qc * 512)
                    ov = ps.tile([D + 1, 2, 512], f32, tag="ov", bufs=1, name="ov")
                    pend = []

                    def emit_out(jc, es, nj):
                        for h in range(2):
                            nc.tensor.matmul(
                                ov[:, h, :nq],
                                v_nat[0:nj, gb + jc, h, :],
                                es[:nj, h, :nq],
                                start=(jc == 0),
                                stop=(jc == nk - 1),
                            )

                    for jc in range(nk):
                        feed_cb()
                        nj = min(128, L - jc * 128)
                        k0 = O + jc * 128
                        sps = ps.tile(
                            [128, 2, 512], f32, tag="sc", bufs=2, name="sps"
                        )
                        # two heads on PE row tiles (0,0) / (64,0): concurrent
                        for h in range(2):
                            p0 = D * h
                            nc.tensor.matmul(
                                sps[:nj, h, :nq],
                                kT[p0 : p0 + D, k0 : k0 + nj],
                                qT[p0 : p0 + D, q0 : q0 + nq],
                                start=True,
                                stop=True,
                            )
                        es = esp.tile([128, 2, 512], bf16, tag="es", name="es")
                        nc.scalar.activation(
                            es[:nj, :, :nq], sps[:nj, :, :nq], Exp, scale=0.125
                        )
                        pend.append((jc, es, nj))
                        if len(pend) > 2:
                            emit_out(*pend.pop(0))
                    for p in pend:
                        emit_out(*p)
                    ot = otp.tile([D + 1, 2, 512], f32, tag="ot", name="ot")
                    nc.vector.tensor_copy(ot[:, :, :nq], ov[:, :, :nq])
                    nc.sync.dma_start(
                        out[:, q0 : q0 + nq].rearrange("(h p) n -> p h n", p=65),
                        ot[:, :, :nq],
                    )

            # --- interleaved emission: chunks back-to-front via the feeder;
            # a unit is ready once all chunks covering [O, O+L) are emitted.
            pending = sorted(units, key=lambda u: u[0], reverse=True)
            for u in pending:
                while state["ti_next"] * 512 > u[0]:
                    feed(1)
                emit_attention(*u)
            feed(n_groups)  # drain any leftovers

    nc.compile()
    return nc


def _prepare(hidden_states, Wqkv_weight, Wqkv_bias, cu_seqlens):
    """Host-side sharding prep. Returns (nc, in_maps, meta)."""
    hs = np.asarray(hidden_states, dtype=np.float32)
    W = np.asarray(Wqkv_weight, dtype=np.float32)
    b = np.asarray(Wqkv_bias, dtype=np.float32).reshape(-1)
    cs = np.asarray(cu_seqlens).astype(np.int64).reshape(-1)
    nnz, dim = hs.shape
    assert dim == DIM and W.shape == (3 * DIM, DIM)
    lengths = tuple(int(cs[i + 1] - cs[i]) for i in range(len(cs) - 1))
    assert sum(lengths) == nnz, (lengths, nnz)

    units, nnz_pad = _padded_units(lengths)
    nc = _build(nnz_pad, units)

    # padded token index map: real token t -> padded column pad_idx[t]
    pad_idx = np.empty(nnz, dtype=np.int64)
    t = 0
    for (O, L) in units:
        pad_idx[t : t + L] = np.arange(O, O + L)
        t += L

    xt_pad = np.zeros((DIM, nnz_pad), dtype=np.float32)
    xt_pad[:, pad_idx] = hs.T
    n_ti = nnz_pad // 512
    xt_np = np.ascontiguousarray(
        xt_pad.reshape(DIM // 128, 128, n_ti, 512).transpose(1, 2, 0, 3)
    ).astype(ml_dtypes.bfloat16)

    in_maps = []
    for c in range(N_CORES):
        r0 = c * HEADS_PER_CORE * HEAD_DIM  # 128c
        Wq = W[r0 : r0 + 128, :]
        Wk = W[DIM + r0 : DIM + r0 + 128, :]
        Wv = W[2 * DIM + r0 : 2 * DIM + r0 + 128, :]
        wqk_np = np.ascontiguousarray(
            np.concatenate([Wq.T, Wk.T], axis=1).reshape(DIM // 128, 128, 256)
        ).astype(ml_dtypes.bfloat16)
        wv_np = np.ascontiguousarray(
            Wv.T.reshape(DIM // 128, 128, 128)
        ).astype(ml_dtypes.bfloat16)
        bias_np = np.ascontiguousarray(
            np.stack([b[r0 : r0 + 128], b[DIM + r0 : DIM + r0 + 128]], axis=1)
        )
        in_maps.append(
            {"xt": xt_np, "wqk": wqk_np, "wv": wv_np, "bias": bias_np}
        )
    meta = {"pad_idx": pad_idx, "nnz": nnz, "bv": b[2 * DIM :]}
    return nc, in_maps, meta


def _postprocess(raws, meta):
    """raws: list of per-core [130, nnz_pad] f32 -> full (nnz, 1024) f32."""
    pad_idx = meta["pad_idx"]
    nnz = meta["nnz"]
    bv = meta["bv"]
    out = np.empty((nnz, DIM), dtype=np.float32)
    for c in range(N_CORES):
        raw = np.asarray(raws[c], dtype=np.float32)
        for h in range(HEADS_PER_CORE):
            num = raw[65 * h : 65 * h + 64][:, pad_idx]  # (64, nnz)
            den = raw[65 * h + 64][pad_idx]  # (nnz,)
            col = c * 128 + h * 64
            out[:, col : col + 64] = num.T / den[:, None] + bv[col : col + 64]
    return out


def kernel(hidden_states, Wqkv_weight, Wqkv_bias, cu_seqlens, max_seqlen=None):
    from concourse.bass_utils import run_bass_kernel_spmd

    nc, in_maps, meta = _prepare(
        hidden_states, Wqkv_weight, Wqkv_bias, cu_seqlens
    )
    res = run_bass_kernel_spmd(nc, in_maps, list(range(N_CORES)))
    return _postprocess(
        [res.results[c]["out"] for c in range(N_CORES)], meta
    )


# revision 7
# speedup vs baseline: 1.4395x; 1.0524x over previous
"""Trainium2 Bass kernel for varlen (ragged) BERT self-attention.

Strategy: tensor-parallel over heads. 16 heads across 8 NeuronCores ->
2 heads per core. Every core runs an IDENTICAL program (SPMD) on:
  - xt:   full hidden_states, host-padded so each sequence starts at a
          128-aligned token offset, transposed + bf16, laid out
          [128, n_ti, 8, 512] so each 512-token chunk is one contiguous
          8KB-per-partition DMA read.
  - wqk:  this core's q/k weight slices as matmul-lhsT blocks
          [8, 128, 256] bf16 (cols 0:128 = q, 128:256 = k).
  - wv:   this core's v weight slice as matmul-rhs blocks [8, 128, 128].
  - bias: q/k bias [128, 2] f32 (per-partition adds on eviction).
Output per core: raw [130, nnz_pad] f32: rows 0:64   = sum_k p*v (head0)
                                         row  64    = sum_k p   (head0)
                                         rows 65:129/129 same for head1.
Host divides by the denominator row, adds the v bias (algebraically
exact: softmax(s) @ (v+bv) = softmax(s)@v + bv), and gathers the
padded layout back to the ragged one.

On-chip per core:
  1. QKV projection, K=1024 in 8 chunks of 128:
     - q,k computed TRANSPOSED (stationary = W slices): qT/kT
       [128(=2h x 64d), nnz_pad] bf16, bias added during PSUM->SBUF
       eviction (DVE tensor_scalar add, per-partition).
     - v computed NATURAL (stationary = xt chunks, moving = Wv):
       v_nat [128(tok), chunk, 2, 65] bf16 with a preset ones column;
       no transposes needed anywhere.
  2. Attention per sequence (no packing; every sequence 128-aligned):
     scores[k,q] = kT.T @ qT per head with K=64 -> the two heads'
     matmuls auto-place on PE row tiles (0,0)/(64,0) and run
     CONCURRENTLY. exp on ACT over both heads in one instruction
     (scale=1/8 folded in). outT[65, q] accumulated as
     (v|ones).T @ exp with K<=128; the ones column gives the softmax
     denominator for free. PSUM -> SBUF copy (DVE) -> DMA out raw.

Emission interleaves per-jc attention into the QKV chunk stream
(chunks processed back-to-front, sequences emitted largest-first) so
the PE instruction stream stays dense end-to-end (HAM clock gate).
"""

import functools
import sys

import numpy as np

for _p in ("/opt/trn_rl_repo",):
    if _p not in sys.path:
        sys.path.append(_p)

import ml_dtypes  # noqa: E402

N_HEADS = 16
HEAD_DIM = 64
DIM = 1024
N_CORES = 8
HEADS_PER_CORE = N_HEADS // N_CORES  # 2


def _padded_units(lengths):
    """One unit per sequence at a 128-aligned padded offset."""
    units = []
    off = 0
    for L in lengths:
        if L == 0:
            continue
        units.append((off, L))
        off += ((L + 127) // 128) * 128
    nnz_pad = ((off + 511) // 512) * 512
    return tuple(units), nnz_pad


@functools.lru_cache(maxsize=4)
def _build(nnz_pad, units):
    """Build + compile the SPMD Bass program for the given ragged lengths."""
    import concourse.mybir as mybir
    import concourse.tile as tile
    from concourse import bacc

    f32 = mybir.dt.float32
    bf16 = mybir.dt.bfloat16
    Exp = mybir.ActivationFunctionType.Exp

    KC = DIM // 128  # 8 contraction chunks
    D = HEAD_DIM
    n_ti = nnz_pad // 512
    n_ch = nnz_pad // 128

    nc = bacc.Bacc("TRN2", target_bir_lowering=False, debug=False)
    xt = nc.declare_dram_parameter("xt", [128, n_ti, KC, 512], bf16, isOutput=False)
    wqk = nc.declare_dram_parameter("wqk", [KC, 128, 256], bf16, isOutput=False)
    wv = nc.declare_dram_parameter("wv", [KC, 128, 128], bf16, isOutput=False)
    bias = nc.declare_dram_parameter("bias", [128, 2], f32, isOutput=False)
    out = nc.declare_dram_parameter("out", [130, nnz_pad], f32, isOutput=True)

    with tile.TileContext(nc) as tc:
        with (
            tc.tile_pool(name="res", bufs=1) as res,
            tc.tile_pool(name="xp", bufs=4) as xp,
            tc.tile_pool(name="esp", bufs=26) as esp,
            tc.tile_pool(name="otp", bufs=4) as otp,
            tc.tile_pool(name="ps", bufs=1, space="PSUM") as ps,
        ):
            # --- resident tensors ---
            wqk_sb = res.tile([128, KC, 256], bf16)
            nc.sync.dma_start(wqk_sb[:], wqk[:, :, :].rearrange("a p m -> p a m"))
            wv_sb = res.tile([128, KC, 128], bf16)
            nc.sync.dma_start(wv_sb[:], wv[:, :, :].rearrange("a p m -> p a m"))
            bias_sb = res.tile([128, 2], f32)
            nc.sync.dma_start(bias_sb[:], bias[:, :])

            qT = res.tile([128, nnz_pad], bf16)
            kT = res.tile([128, nnz_pad], bf16)
            # v in natural layout: [tok(128), chunk, head, 64 v + 1 ones]
            v_nat = res.tile([128, n_ch, 2, D + 1], bf16)
            nc.gpsimd.memset(v_nat[:, :, :, D : D + 1], 1.0)

            # --- QKV feeder: yields 3 groups per token chunk (q, k+v0, v1)
            # so attention emission can interleave dense PE work ---
            state = {"ti_next": n_ti}

            def _qkv_groups():
                for ti in range(n_ti - 1, -1, -1):
                    t0 = ti * 512
                    xt_tile = xp.tile([128, KC, 512], bf16, tag="xt", name="xt_t")
                    nc.sync.dma_start(xt_tile[:], xt[:, ti, :, :])
                    # q group: stationary W, moving xt
                    mmq = ps.tile([128, 512], f32, tag="mm", bufs=2, name="mmq")
                    for kc in range(KC):
                        nc.tensor.matmul(
                            mmq[:, :],
                            wqk_sb[:, kc, 0:128],
                            xt_tile[:, kc, :],
                            start=(kc == 0),
                            stop=(kc == KC - 1),
                        )
                    nc.vector.tensor_scalar_add(
                        qT[:, t0 : t0 + 512], mmq[:, :], bias_sb[:, 0:1]
                    )
                    yield
                    # k group interleaved with first half of v (v: stationary
                    # xt chunk, moving Wv -> LDW-heavy; hide under k streams)
                    mmk = ps.tile([128, 512], f32, tag="mm", bufs=2, name="mmk")
                    mmv = ps.tile([128, 4, 2, D], f32, tag="mm", bufs=2, name="mmv")
                    vs = [(c, kc) for c in range(4) for kc in range(KC)]
                    vi = 0

                    def _vmm(c, kc):
                        nc.tensor.matmul(
                            mmv[:, c, :, :],
                            xt_tile[:, kc, c * 128 : (c + 1) * 128],
                            wv_sb[:, kc, :],
                            start=(kc == 0),
                            stop=(kc == KC - 1),
                        )

                    for kc in range(KC):
                        nc.tensor.matmul(
                            mmk[:, :],
                            wqk_sb[:, kc, 128:256],
                            xt_tile[:, kc, :],
                            start=(kc == 0),
                            stop=(kc == KC - 1),
                        )
                        for _ in range(2):
                            _vmm(*vs[vi])
                            vi += 1
                    nc.vector.tensor_scalar_add(
                        kT[:, t0 : t0 + 512], mmk[:, :], bias_sb[:, 1:2]
                    )
                    yield
                    # rest of v
                    while vi < len(vs):
                        _vmm(*vs[vi])
                        vi += 1
                    nc.vector.tensor_copy(
                        v_nat[:, ti * 4 : ti * 4 + 4, :, 0:D], mmv[:, :, :, :]
                    )
                    state["ti_next"] = ti
                    yield

            feeder = _qkv_groups()

            # pacing: spread remaining feeder groups over remaining
            # attention jc-iterations (recomputed each step)
            n_groups = 3 * n_ti
            n_iters = sum(
                ((L + 127) // 128) * ((L + 511) // 512) for _, L in units
            )
            pace = {"acc": 0.0, "groups": n_groups, "iters": n_iters}

            def feed(n):
                for _ in range(n):
                    if next(feeder, "done") == "done":
                        break
                    pace["groups"] -= 1

            def feed_cb():
                if pace["iters"] > 0:
                    pace["acc"] += pace["groups"] / pace["iters"]
                pace["iters"] -= 1
                k = min(int(pace["acc"]), pace["groups"])
                if k > 0:
                    pace["acc"] -= k
                    feed(k)
                elif pace["groups"] == 0:
                    # feeder dry: PE keepalive so the HAM clock gate stays
                    # released through ACT-bound stretches
                    for _ in range(2):
                        dm = ps.tile([128, 512], f32, tag="mm", bufs=2, name="dm")
                        nc.tensor.matmul(
                            dm[:, :],
                            wqk_sb[:, 0, 0:128],
                            qT[:, 0:512],
                            start=True,
                            stop=True,
                        )

            # --- deferred out-matmul machinery -------------------------------
            # Out matmuls run as 64-row-mode split-K pairs (tokens 0:64 on PE
            # row tile (0,0), tokens 64:128 on (64,0)) so the whole attention
            # stream stays in one PE tiling mode (mode switches cost ~300ns).
            # Each (qc, head) package accumulates into ov[:, 0/1, :] (a/b
            # banks); the a+b merge happens in the DVE eviction for free.
            # Packages are queued and drained during the FOLLOWING qc's score
            # slots, overlapping the ACT-paced exp stream.
            outq = []  # pending out-pairs: (emit_fn,)

            def drain_out(k):
                while k > 0 and outq:
                    outq.pop(0)()
                    k -= 1

            def push_pkg(O, L, q0, nq, es_list, ot_cell):
                """Queue out-matmuls + eviction for one (qc); both heads."""
                gb = O // 128
                nk = len(es_list)
                jbs = [jc for jc, (_, nj) in enumerate(es_list) if nj > 64]
                for h in range(2):
                    ov = ps.tile(
                        [D + 1, 2, 512], f32, tag="ov", bufs=1, name="ov"
                    )

                    def mk(jc, es, nj, h=h, ov=ov):
                        def go():
                            na = min(nj, 64)
                            nc.tensor.matmul(
                                ov[:, 0, :nq],
                                v_nat[0:na, gb + jc, h, :],
                                es[0:na, h, :nq],
                                start=(jc == 0),
                                stop=(jc == nk - 1),
                            )
                            if nj > 64:
                                nc.tensor.matmul(
                                    ov[:, 1, :nq],
                                    v_nat[64:nj, gb + jc, h, :],
                                    es[64:nj, h, :nq],
                                    start=(jc == jbs[0]),
                                    stop=(jc == jbs[-1]),
                                )

                        return go

                    for jc, (es, nj) in enumerate(es_list):
                        outq.append(mk(jc, es, nj))

                    def evict(h=h, ov=ov):
                        if ot_cell[0] is None:
                            ot_cell[0] = otp.tile(
                                [D + 1, 2, 512], f32, tag="ot", name="ot"
                            )
                        ot = ot_cell[0]
                        nc.vector.tensor_copy(ot[:, h, :nq], ov[:, 0, :nq])
                        if jbs:
                            nc.vector.tensor_add(
                                ot[:, h, :nq], ot[:, h, :nq], ov[:, 1, :nq]
                            )
                        if h == 1:
                            nc.sync.dma_start(
                                out[:, q0 : q0 + nq].rearrange(
                                    "(h p) n -> p h n", p=65
                                ),
                                ot[:, :, :nq],
                            )

                    outq.append(evict)

            def emit_attention(O, L):
                nk = (L + 127) // 128
                for qc in range((L + 511) // 512):
                    q0 = O + qc * 512
                    nq = min(512, L - qc * 512)
                    es_list = []
                    for jc in range(nk):
                        feed_cb()
                        nj = min(128, L - jc * 128)
                        k0 = O + jc * 128
                        sps = ps.tile(
                            [128, 2, 512], f32, tag="sc", bufs=2, name="sps"
                        )
                        # two heads on PE row tiles (0,0) / (64,0): concurrent
                        for h in range(2):
                            p0 = D * h
                            nc.tensor.matmul(
                                sps[:nj, h, :nq],
                                kT[p0 : p0 + D, k0 : k0 + nj],
                                qT[p0 : p0 + D, q0 : q0 + nq],
                                start=True,
                                stop=True,
                            )
                        es = esp.tile([128, 2, 512], bf16, tag="es", name="es")
                        nc.scalar.activation(
                            es[:nj, :, :nq], sps[:nj, :, :nq], Exp, scale=0.125
                        )
                        es_list.append((es, nj))
                        drain_out(2)
                    push_pkg(O, L, q0, nq, es_list, [None])

            # --- interleaved emission: chunks back-to-front via the feeder;
            # a unit is ready once all chunks covering [O, O+L) are emitted.
            pending = sorted(units, key=lambda u: u[0], reverse=True)
            for u in pending:
                while state["ti_next"] * 512 > u[0]:
                    feed(1)
                emit_attention(*u)
            drain_out(1 << 30)
            feed(n_groups)  # drain any leftovers

    nc.compile()
    return nc


def _prepare(hidden_states, Wqkv_weight, Wqkv_bias, cu_seqlens):
    """Host-side sharding prep. Returns (nc, in_maps, meta)."""
    hs = np.asarray(hidden_states, dtype=np.float32)
    W = np.asarray(Wqkv_weight, dtype=np.float32)
    b = np.asarray(Wqkv_bias, dtype=np.float32).reshape(-1)
    cs = np.asarray(cu_seqlens).astype(np.int64).reshape(-1)
    nnz, dim = hs.shape
    assert dim == DIM and W.shape == (3 * DIM, DIM)
    lengths = tuple(int(cs[i + 1] - cs[i]) for i in range(len(cs) - 1))
    assert sum(lengths) == nnz, (lengths, nnz)

    units, nnz_pad = _padded_units(lengths)
    nc = _build(nnz_pad, units)

    # padded token index map: real token t -> padded column pad_idx[t]
    pad_idx = np.empty(nnz, dtype=np.int64)
    t = 0
    for (O, L) in units:
        pad_idx[t : t + L] = np.arange(O, O + L)
        t += L

    xt_pad = np.zeros((DIM, nnz_pad), dtype=np.float32)
    xt_pad[:, pad_idx] = hs.T
    n_ti = nnz_pad // 512
    xt_np = np.ascontiguousarray(
        xt_pad.reshape(DIM // 128, 128, n_ti, 512).transpose(1, 2, 0, 3)
    ).astype(ml_dtypes.bfloat16)

    in_maps = []
    for c in range(N_CORES):
        r0 = c * HEADS_PER_CORE * HEAD_DIM  # 128c
        Wq = W[r0 : r0 + 128, :]
        Wk = W[DIM + r0 : DIM + r0 + 128, :]
        Wv = W[2 * DIM + r0 : 2 * DIM + r0 + 128, :]
        wqk_np = np.ascontiguousarray(
            np.concatenate([Wq.T, Wk.T], axis=1).reshape(DIM // 128, 128, 256)
        ).astype(ml_dtypes.bfloat16)
        wv_np = np.ascontiguousarray(
            Wv.T.reshape(DIM // 128, 128, 128)
        ).astype(ml_dtypes.bfloat16)
        bias_np = np.ascontiguousarray(
            np.stack([b[r0 : r0 + 128], b[DIM + r0 : DIM + r0 + 128]], axis=1)
        )
        in_maps.append(
            {"xt": xt_np, "wqk": wqk_np, "wv": wv_np, "bias": bias_np}
        )
    meta = {"pad_idx": pad_idx, "nnz": nnz, "bv": b[2 * DIM :]}
    return nc, in_maps, meta


def _postprocess(raws, meta):
    """raws: list of per-core [130, nnz_pad] f32 -> full (nnz, 1024) f32."""
    pad_idx = meta["pad_idx"]
    nnz = meta["nnz"]
    bv = meta["bv"]
    out = np.empty((nnz, DIM), dtype=np.float32)
    for c in range(N_CORES):
        raw = np.asarray(raws[c], dtype=np.float32)
        for h in range(HEADS_PER_CORE):
            num = raw[65 * h : 65 * h + 64][:, pad_idx]  # (64, nnz)
            den = raw[65 * h + 64][pad_idx]  # (nnz,)
            col = c * 128 + h * 64
            out[:, col : col + 64] = num.T / den[:, None] + bv[col : col + 64]
    return out


def kernel(hidden_states, Wqkv_weight, Wqkv_bias, cu_seqlens, max_seqlen=None):
    from concourse.bass_utils import run_bass_kernel_spmd

    nc, in_maps, meta = _prepare(
        hidden_states, Wqkv_weight, Wqkv_bias, cu_seqlens
    )
    res = run_bass_kernel_spmd(nc, in_maps, list(range(N_CORES)))
    return _postprocess(
        [res.results[c]["out"] for c in range(N_CORES)], meta
    )
